# revision 45
# baseline (speedup 1.0000x reference)
# Trainium2 Bass kernel for single-head attention:
#   out = softmax((q@Wq+bq)(k@Wk+bk)^T / sqrt(D)) @ (v@Wv+bv) @ Wo + bo
# Full shapes: query/key/value [4, 2048, 1024], D=1024, mask all-ones.
#
# Sharding: data-parallel over (batch, query-half) -> 8 shards, one per
# NeuronCore. Core c handles batch b=c//2, query rows [h*1024, (h+1)*1024)
# with h=c%2. Each core projects only its OWN half of the batch's key/value
# tokens. The k-axis is PERMUTED per core: own tokens occupy k-positions
# [0, 1024) ("half A"), partner tokens [1024, 2048) ("half B") — softmax and
# P@V are permutation-invariant as long as K and V use the same order, so
# the SPMD program stays rank-independent. The partner half is obtained
# with a pairwise AllReduce(add) of the bf16 halves plus an on-chip
# subtract (partner = sum - own), which keeps every access pattern static.
#
# KEY FUSIONS (host-side exact fp32 weight algebra, device work deleted):
# 1. Output projection folded into V:
#      (P @ (v Wv)) @ Wo == P @ (v (Wv Wo)),  Wvo = Wv@Wo free on host.
# 2. K projection folded into Q:
#      (q Wq + bq)·(k Wk + bk) = q (Wq Wk^T) k^T + [softmax-invariant terms]
#    W' = (Wq Wk^T)/sqrt(D) on host; scores contract RAW k against
#    Q' = q@W'; leftover per-k bias b3[k] rides the Exp eviction's ACT bias.
#    This lets the K exchange run on RAW input data, so the collective
#    fires with no compute dependency at all.
#
# LAYOUT: everything feature-major; ALL host inputs are PARTITION-MAJOR
# [128, dt*cols] so every DMA descriptor is a contiguous >=2KB
# per-partition line (the naive [D, T] layout yields 512B descriptors and
# the DMA system crawls at ~30GB/s; partition-major sustains HW-DGE rate).
#   qx/kx/vx [P, DT*TQ]   (x[p, dt*TQ+t] = x_T[dt*128+p, t]),  bf16
#   wq/wv    [P, DT*D]    (w[p, dt*D+n] = W[dt*128+p, n]),     bf16
# Q' projection runs dt-OUTER (accumulating 4 o-banks per (och, tc) combo,
# tc-outer) so compute starts once the first per-dt slab pair (~0.5MB)
# lands instead of the full 4MB.
# Scores^T[k,q] accumulate over d (lhsT = raw-k tile, rhs = Q' block); Exp
# evicts P^T unnormalized; 1/rowsum applied as per-partition ACT scale on
# the final eviction. Row sums: DVE accumulates k-tile slabs into s1, one
# tiny PE matmul per 128-token group reduces over partitions.
# The attention loop is software-pipelined: all partner-independent half-A
# work is emitted before any half-B work, covering the collectives.
#
# DMA-QUEUE PLAN (each dma_start on a HW-DGE ring costs ~0.65us of
# sequencer descriptor-gen + a ~2-3us serialized completion-receipt
# bubble, and the rings ramp slowly for the first ~15us, so BOTH the DMA
# count and which ring carries what were tuned empirically — many
# theoretically-prettier plans measured slower):
#   sync (SP HW-DGE):    wq/qx dt-pair slabs (alternating) -> vx ->
#                        recovery -> output
#   scalar (ACT HW-DGE): wq/qx dt-pair slabs (alternating) -> wv ->
#                        kT bounce -> evictions
#   gpsimd (SW-DGE):     b3 -> raw-K own half; + the two collective
#                        triggers (nothing compute-critical parks behind
#                        a collective)
# The Q'-phase inputs (wq+qx, 4MB) alternate dt-pair slabs across BOTH
# rings in matched dt order, so each ring carries 2MB and the last slab
# lands ~10us earlier than a ring-per-tensor plan (first MM ~15us, Q'
# stalls ~7us vs ~20us/~9us before).
# Output is bf16, one full-D DMA per 128-token tile (2KB rows, half the
# receipt bubbles); host upcasts and adds bo' = bv@Wo + bo (softmax rows
# sum to 1).
#
# Tried and rejected: fp8/DoubleRow everywhere (softmax-weighted averaging
# passes element noise ~1:1 to the output; every fp8 placement measured
# 2.7e-2..8.6e-2 rel err vs the 2e-2 gate in numpy pipeline sims),
# direct collective on the kx input (walrus verifier rejects ExternalInput
# collectives), HAM warm-up dummy matmuls (net-negative on hardware),
# cross-ring half splits and single-DMA-per-tensor plans (slower).

import functools

import ml_dtypes
import numpy as np

B, S, D = 4, 2048, 1024
N_CORES = 8
P = 128
DT = D // P        # 8 d-tiles of 128
TQ = S // 2        # 1024 query rows / kv-half rows per core
NQ = TQ // P       # 8 q-tiles
NK = S // P        # 16 k-tiles
NKH = NK // 2      # 8 k-tiles per half
SCALE = 1.0 / np.sqrt(np.float32(D))  # 1/32
BF16 = ml_dtypes.bfloat16
PAIRS = [[0, 1], [2, 3], [4, 5], [6, 7]]


@functools.lru_cache(maxsize=1)
def _build():
    import concourse.bass as bass  # noqa: F401  (registers engines)
    import concourse.mybir as mybir
    import concourse.tile as tile
    from concourse import bacc

    f32 = mybir.dt.float32
    bf16 = mybir.dt.bfloat16

    nc = bacc.Bacc("TRN2", target_bir_lowering=False, debug=False,
                   num_devices=N_CORES)

    qx = nc.dram_tensor("qx", [P, DT * TQ], bf16, kind="ExternalInput")
    kx = nc.dram_tensor("kx", [P, DT * TQ], bf16, kind="ExternalInput")
    vx = nc.dram_tensor("vx", [P, DT * TQ], bf16, kind="ExternalInput")
    wq = nc.dram_tensor("wq", [P, DT * D], bf16, kind="ExternalInput")
    wv = nc.dram_tensor("wv", [P, DT * D], bf16, kind="ExternalInput")
    b3_d = nc.dram_tensor("b3", [P, NK], f32, kind="ExternalInput")
    out_d = nc.dram_tensor("out", [TQ, D], bf16, kind="ExternalOutput")

    Ident = mybir.ActivationFunctionType.Identity
    Exp = mybir.ActivationFunctionType.Exp

    with tile.TileContext(nc) as tc:
        with (
            tc.tile_pool(name="const", bufs=1) as const,
            tc.tile_pool(name="wpool", bufs=2) as wpool,
            tc.tile_pool(name="big", bufs=1) as big,
            tc.tile_pool(name="work", bufs=2) as work,
            tc.tile_pool(name="sums", bufs=1) as sums,
            tc.tile_pool(name="ptp", bufs=2) as ptp,
            tc.tile_pool(name="dram", bufs=1, space="DRAM") as dram,
            tc.tile_pool(name="mmps", bufs=4, space="PSUM") as mmps,
            tc.tile_pool(name="scps", bufs=3, space="PSUM") as scps,
            tc.tile_pool(name="rsps", bufs=1, space="PSUM") as rsps,
        ):
            qx_ap = qx.ap().rearrange("p (dt t) -> p dt t", dt=DT)
            kx_ap = kx.ap().rearrange("p (dt t) -> p dt t", dt=DT)
            vx_ap = vx.ap().rearrange("p (dt t) -> p dt t", dt=DT)
            wq_ap = wq.ap().rearrange("p (dt n) -> p dt n", dt=DT)
            wv_ap = wv.ap().rearrange("p (dt n) -> p dt n", dt=DT)

            # ---- constants ----
            ones_sb = const.tile([P, 1], bf16, tag="ones")
            nc.vector.memset(ones_sb[:], 1.0)
            rs_ps = rsps.tile([P, NQ], f32, tag="rs")
            b3_sb = const.tile([P, NK], f32, tag="b3")
            # b3 rides gpsimd: every dma on a HW-DGE ring costs ~2-3us of
            # serialized completion-receipt, so the scalar ring head is
            # reserved for wq (which gates the first matmul).
            nc.gpsimd.dma_start(b3_sb[:], b3_d.ap())
            r_all = const.tile([P, NQ], f32, tag="rall")

            # ---- persistent intermediates ----
            QT = big.tile([P, DT, TQ], bf16, tag="QT")       # 2 MB  Q' = q@W'
            KTr = big.tile([P, DT, S], bf16, tag="KTr")      # 4 MB raw k, A+B
            Vtm = big.tile([P, NK, D], bf16, tag="Vtm")      # 4 MB token-major V'
            aoM = big.tile([P, NQ, D], bf16, tag="aoM")      # 2 MB attn_out half-A
            qxt = big.tile([P, DT, TQ], bf16, tag="qxt")     # 2 MB raw q slabs
            vxt = big.tile([P, DT, TQ], bf16, tag="vxt")     # 2 MB raw v slabs

            # ---- collectives: one 2MB AllReduce per tensor. K exchanges
            # RAW input data — a DRAM->DRAM bounce of the kx input (the
            # walrus verifier rejects collectives reading ExternalInputs
            # directly) rides the scalar queue tail so its 2MB never
            # blocks the sync descriptor ring the first matmuls wait on.
            ex_k_in = dram.tile([P, DT * TQ], bf16)
            ex_k_out = dram.tile([P, DT * TQ], bf16)
            ex_v_in = dram.tile([P, NKH, D], bf16)
            ex_v_out = dram.tile([P, NKH, D], bf16)

            # ---- weights/input streams: the empirically-tuned queue plan
            # from the header. qx slabs ride sync and wq slabs ride scalar
            # so the two HW-DGE rings deliver the Q'-phase inputs in
            # parallel; slab sizes 0.25/0.75/1MB balance earliest-start
            # (small first slab) against per-DMA receipt bubbles (few
            # DMAs). All slices are >=2KB-contiguous per partition.
            w_qk = wpool.tile([P, DT, D], bf16, tag="w")
            w_v = wpool.tile([P, DT, D], bf16, tag="w")
            # dt-pair slabs of wq/qx alternate across BOTH rings in
            # matched dt order, so each ring carries only 2MB of the
            # Q'-phase inputs and the last slab lands ~10us earlier than
            # a single-ring-per-tensor plan.
            nc.scalar.dma_start(w_qk[:, 0:2, :], wq_ap[:, 0:2, :])
            nc.sync.dma_start(qxt[:, 0:2, :], qx_ap[:, 0:2, :])
            nc.sync.dma_start(w_qk[:, 2:4, :], wq_ap[:, 2:4, :])
            nc.scalar.dma_start(qxt[:, 2:4, :], qx_ap[:, 2:4, :])
            nc.scalar.dma_start(w_qk[:, 4:6, :], wq_ap[:, 4:6, :])
            nc.sync.dma_start(qxt[:, 4:6, :], qx_ap[:, 4:6, :])
            nc.sync.dma_start(w_qk[:, 6:8, :], wq_ap[:, 6:8, :])
            nc.scalar.dma_start(qxt[:, 6:8, :], qx_ap[:, 6:8, :])
            nc.scalar.dma_start(w_v[:, :4, :], wv_ap[:, :4, :])
            nc.scalar.dma_start(w_v[:, 4:, :], wv_ap[:, 4:, :])
            nc.scalar.dma_start(ex_k_in[:], kx.ap())
            nc.sync.dma_start(vxt[:, :4, :], vx_ap[:, :4, :])
            nc.sync.dma_start(vxt[:, 4:, :], vx_ap[:, 4:, :])
            # raw K own half rides the sync TAIL: the head window is
            # device-HBM-bound (8 cores streaming at once), so the 2MB
            # KTr load — not needed until the first scores (~70us) —
            # defers past the congestion where the ring runs fast again.
            nc.sync.dma_start(KTr[:, :, 0:TQ], kx_ap[:, :, :])
            nc.gpsimd.collective_compute(
                "AllReduce", mybir.AluOpType.add, replica_groups=PAIRS,
                ins=[ex_k_in.opt()], outs=[ex_k_out.opt()],
            )

            # ---- Q' projection, dt-OUTER: for each (o-chunk, t-chunk)
            # combo, accumulate all 8 dt slabs into 4 PSUM banks. The
            # first matmul needs only the dt0 slab pair (~0.5MB landed).
            for tc_i in range(2):
                for och in range(2):
                    tsl = slice(tc_i * 512, (tc_i + 1) * 512)
                    pss = [mmps.tile([P, 512], f32, tag="mm",
                                     name=f"qp{och}{tc_i}{oi}")
                           for oi in range(4)]
                    for dt_i in range(DT):
                        for oi in range(4):
                            o = och * 4 + oi
                            nc.tensor.matmul(
                                pss[oi][:],
                                w_qk[:, dt_i, o * P:(o + 1) * P],
                                qxt[:, dt_i, tsl],
                                start=(dt_i == 0),
                                stop=(dt_i == DT - 1),
                            )
                    for oi in range(4):
                        nc.scalar.copy(QT[:, och * 4 + oi, tsl], pss[oi][:])

            # ---- V projection (own half, token-major -> Vtm[:, 0:8, :]) ----
            def v_proj():
                for c in range(TQ // 512):
                    for sub in range(4):        # 4 tok-tiles per chunk
                        tt = c * 4 + sub
                        for dc in range(2):     # dout chunks of 512
                            ps = mmps.tile([P, 512], f32, tag="mm")
                            for dt_i in range(DT):
                                nc.tensor.matmul(
                                    ps[:],
                                    vxt[:, dt_i, tt * P:(tt + 1) * P],
                                    w_v[:, dt_i, dc * 512:(dc + 1) * 512],
                                    start=(dt_i == 0),
                                    stop=(dt_i == DT - 1),
                                )
                            nc.scalar.copy(
                                Vtm[:, tt, dc * 512:(dc + 1) * 512], ps[:])
                    # stream the finished 4-tile group back in one DMA
                    nc.scalar.dma_start(ex_v_in[:, c * 4:(c + 1) * 4, :],
                                        Vtm[:, c * 4:(c + 1) * 4, :])
                nc.gpsimd.collective_compute(
                    "AllReduce", mybir.AluOpType.add, replica_groups=PAIRS,
                    ins=[ex_v_in.opt()], outs=[ex_v_out.opt()],
                )

            # partner-half recovery: DMA the pair-sum into the B-half
            # (sync queue tail), subtract own in place on the DVE with LATE
            # priority so the scheduler never parks these at the DVE queue
            # head in front of the s1/accumulation chains.
            ex_k_ap = ex_k_out[:].rearrange("p (dt t) -> p dt t", dt=DT)

            def sub_k(c):
                dst = KTr[:, :, TQ + c * 512:TQ + (c + 1) * 512]
                nc.sync.dma_start(dst, ex_k_ap[:, :, c * 512:(c + 1) * 512])
                with tc.high_priority(offset=-100000):
                    nc.vector.tensor_tensor(
                        dst, dst, KTr[:, :, c * 512:(c + 1) * 512],
                        mybir.AluOpType.subtract,
                    )

            def sub_v(c):
                dst = Vtm[:, NKH + c * 4:NKH + (c + 1) * 4, :]
                nc.sync.dma_start(dst, ex_v_out[:, c * 4:(c + 1) * 4, :])
                with tc.high_priority(offset=-100000):
                    nc.vector.tensor_tensor(
                        dst, dst, Vtm[:, c * 4:(c + 1) * 4, :],
                        mybir.AluOpType.subtract,
                    )

            # ---- attention, software-pipelined over 512-wide q-blocks ----
            pT_tiles = {}
            s1_tiles = {}

            def s1_add(blk, kt):
                s1 = s1_tiles[blk]
                if kt == 0:
                    nc.vector.tensor_copy(s1[:], pT_tiles[blk][:, 0, :])
                else:
                    nc.vector.tensor_tensor(
                        s1[:], pT_tiles[blk][:, kt, :], s1[:],
                        mybir.AluOpType.add)

            def score_half(blk, half, defer_sums=False):
                qsl = slice(blk * 512, (blk + 1) * 512)
                pT = pT_tiles[blk]
                for kt in range(half * NKH, half * NKH + NKH):
                    sc = scps.tile([P, 512], f32, tag="sc")
                    for dt_i in range(DT):
                        nc.tensor.matmul(
                            sc[:],
                            KTr[:, dt_i, kt * P:(kt + 1) * P],
                            QT[:, dt_i, qsl],
                            start=(dt_i == 0),
                            stop=(dt_i == DT - 1),
                        )
                    nc.scalar.activation(pT[:, kt, :], sc[:], Exp,
                                         bias=b3_sb[:, kt:kt + 1])
                    if not defer_sums:
                        s1_add(blk, kt)

            def row_sums(blk):
                s1 = s1_tiles[blk]
                for t4 in range(4):
                    nc.tensor.matmul(
                        rs_ps[:, blk * 4 + t4:blk * 4 + t4 + 1],
                        s1[:, t4 * P:(t4 + 1) * P],
                        ones_sb[:, 0:1],
                        start=True, stop=True,
                    )
                nc.vector.reciprocal(r_all[:, blk * 4:(blk + 1) * 4],
                                     rs_ps[:, blk * 4:(blk + 1) * 4])

            def attn_v(blk, half):
                pT = pT_tiles[blk]
                for tt4 in range(4):
                    tt = blk * 4 + tt4
                    fin = None
                    for dc in range(2):
                        if half == 1 and (tt4 * 2 + dc) % 2 == 0:
                            av = scps.tile([P, 512], f32, tag="sc", name="avs")
                        else:
                            av = mmps.tile([P, 512], f32, tag="mm", name="avm")
                        for kt in range(half * NKH, half * NKH + NKH):
                            nc.tensor.matmul(
                                av[:],
                                pT[:, kt, tt4 * P:(tt4 + 1) * P],
                                Vtm[:, kt, dc * 512:(dc + 1) * 512],
                                start=(kt == half * NKH),
                                stop=(kt == half * NKH + NKH - 1),
                            )
                        dsl = slice(dc * 512, (dc + 1) * 512)
                        if half == 0:
                            nc.scalar.copy(aoM[:, tt, dsl], av[:])
                        else:
                            if dc == 0:
                                fin = work.tile([P, D], bf16, tag="fin")
                            wf = work.tile([P, 512], f32, tag="wf")
                            nc.vector.tensor_tensor(
                                wf[:], av[:], aoM[:, tt, dsl],
                                mybir.AluOpType.add,
                            )
                            nc.scalar.activation(
                                fin[:, dsl], wf[:], Ident,
                                scale=r_all[:, tt:tt + 1],
                            )
                            if dc == 1:
                                # one full-D output DMA per token tile:
                                # 2KB-contiguous rows, half the per-DMA
                                # receipt bubbles on the sync ring
                                nc.sync.dma_start(
                                    out_d.ap()[tt * P:(tt + 1) * P, :],
                                    fin[:])

            def p1a(blk):
                pT_tiles[blk] = ptp.tile(
                    [P, NK, 512], bf16, tag="pT", name=f"pT{blk}")
                s1_tiles[blk] = sums.tile([P, 512], bf16, tag=f"s1b{blk}",
                                          name=f"s1b{blk}")
                score_half(blk, 0)

            def p2(blk):
                score_half(blk, 1)

            def p3(blk):
                attn_v(blk, 1)

            v_proj()
            p1a(0)
            attn_v(0, 0)
            p1a(1)
            attn_v(1, 0)
            sub_k(0)
            sub_k(1)
            sub_v(0)
            sub_v(1)
            p2(0)
            p2(1)
            row_sums(0)
            p3(0)
            row_sums(1)
            p3(1)

    nc.compile()
    return nc


def _numpy_reference(query, key, value, mask, Wq, bq, Wk, bk, Wv, bv, Wo, bo):
    q = query @ Wq + bq
    k = key @ Wk + bk
    v = value @ Wv + bv
    s = np.einsum("bsd,btd->bst", q, k) / np.sqrt(np.float32(q.shape[-1]))
    s = np.where(mask == 0, np.float32(-1e9), s)
    s = s - s.max(axis=-1, keepdims=True)
    e = np.exp(s)
    p = e / e.sum(axis=-1, keepdims=True)
    x = np.einsum("bst,btd->bsd", p, v)
    return (x @ Wo + bo).astype(np.float32)


def _pmajor(arr_T):
    # [D, T] feature-major -> partition-major [P, DT*T]
    T = arr_T.shape[1]
    return np.ascontiguousarray(
        arr_T.reshape(DT, P, T).transpose(1, 0, 2).reshape(P, DT * T))


def kernel(query, key, value, mask, Wq, bq, Wk, bk, Wv, bv, Wo, bo):
    query = np.asarray(query, np.float32)
    key = np.asarray(key, np.float32)
    value = np.asarray(value, np.float32)
    mask = np.asarray(mask)
    if not np.all(mask != 0):
        # This problem's mask is always all-ones; keep a correct fallback.
        return _numpy_reference(query, key, value, mask, Wq, bq, Wk, bk,
                                Wv, bv, Wo, bo)

    from concourse.bass_utils import run_bass_kernel_spmd

    nc = _build()

    Wq32 = np.asarray(Wq, np.float32)
    Wk32 = np.asarray(Wk, np.float32)
    # K projection fused into Q: W' = (Wq Wk^T)/sqrt(D), exact in fp32
    wq_b = ((Wq32 @ Wk32.T) * SCALE).astype(BF16)
    # output projection fused into V: V' = v @ (Wv Wo), exact in fp32
    wv_b = (np.asarray(Wv, np.float32) @ np.asarray(Wo, np.float32)
            ).astype(BF16)
    # fusion leftover: per-k score bias b3[k] = k.(Wk bq)/sqrt(D) (the
    # per-q terms cancel in softmax); zero here since bq = 0.
    wkbq = (Wk32 @ np.asarray(bq, np.float32)) * SCALE
    b3_full = np.asarray(key, np.float32) @ wkbq          # [B, S]
    bo_eff = (np.asarray(bv, np.float32) @ np.asarray(Wo, np.float32)
              + np.asarray(bo, np.float32)).astype(np.float32)

    wq_pm = _pmajor(wq_b)
    wv_pm = _pmajor(wv_b)
    in_maps = []
    for c in range(N_CORES):
        b, h = divmod(c, 2)
        sl = slice(h * TQ, (h + 1) * TQ)
        sl_p = slice((1 - h) * TQ, (2 - h) * TQ)
        b3_core = np.concatenate([b3_full[b, sl], b3_full[b, sl_p]])
        in_maps.append({
            "qx": _pmajor(query[b, sl].T.astype(BF16)),
            "kx": _pmajor(key[b, sl].T.astype(BF16)),
            "vx": _pmajor(value[b, sl].T.astype(BF16)),
            "wq": wq_pm, "wv": wv_pm,
            "b3": np.ascontiguousarray(
                b3_core.reshape(NK, P).T.astype(np.float32)),
        })

    global _last_in_maps
    _last_in_maps = in_maps
    res = run_bass_kernel_spmd(nc, in_maps, list(range(N_CORES)))

    out = np.empty((B, S, D), np.float32)
    for c in range(N_CORES):
        b, h = divmod(c, 2)
        out[b, h * TQ:(h + 1) * TQ] = res.results[c]["out"].astype(np.float32)
    out += bo_eff
    return out


# revision 46
# speedup vs baseline: 1.0373x; 1.0373x over previous
# Trainium2 Bass kernel for single-head attention:
#   out = softmax((q@Wq+bq)(k@Wk+bk)^T / sqrt(D)) @ (v@Wv+bv) @ Wo + bo
# Full shapes: query/key/value [4, 2048, 1024], D=1024, mask all-ones.
#
# Sharding: data-parallel over (batch, query-half) -> 8 shards, one per
# NeuronCore. Core c handles batch b=c//2, query rows [h*1024, (h+1)*1024)
# with h=c%2. Each core projects only its OWN half of the batch's key/value
# tokens. The k-axis is PERMUTED per core: own tokens occupy k-positions
# [0, 1024) ("half A"), partner tokens [1024, 2048) ("half B") — softmax and
# P@V are permutation-invariant as long as K and V use the same order, so
# the SPMD program stays rank-independent. The partner half is obtained
# with a pairwise AllReduce(add) of the bf16 halves plus an on-chip
# subtract (partner = sum - own), which keeps every access pattern static.
#
# KEY FUSIONS (host-side exact fp32 weight algebra, device work deleted):
# 1. Output projection folded into V:
#      (P @ (v Wv)) @ Wo == P @ (v (Wv Wo)),  Wvo = Wv@Wo free on host.
# 2. K projection folded into Q:
#      (q Wq + bq)·(k Wk + bk) = q (Wq Wk^T) k^T + [softmax-invariant terms]
#    W' = (Wq Wk^T)/sqrt(D) on host; scores contract RAW k against
#    Q' = q@W'; leftover per-k bias b3[k] rides the Exp eviction's ACT bias.
#    This lets the K exchange run on RAW input data, so the collective
#    fires with no compute dependency at all.
#
# LAYOUT: everything feature-major; ALL host inputs are PARTITION-MAJOR
# [128, dt*cols] so every DMA descriptor is a contiguous >=2KB
# per-partition line (the naive [D, T] layout yields 512B descriptors and
# the DMA system crawls at ~30GB/s; partition-major sustains HW-DGE rate).
#   qx/kx/vx [P, DT*TQ]   (x[p, dt*TQ+t] = x_T[dt*128+p, t]),  bf16
#   wq/wv    [P, DT*D]    (w[p, dt*D+n] = W[dt*128+p, n]),     bf16
# Q' projection runs dt-OUTER (accumulating 4 o-banks per (och, tc) combo,
# tc-outer) so compute starts once the first per-dt slab pair (~0.5MB)
# lands instead of the full 4MB.
# Scores^T[k,q] accumulate over d (lhsT = raw-k tile, rhs = Q' block); Exp
# evicts P^T unnormalized; 1/rowsum applied as per-partition ACT scale on
# the final eviction. Row sums: DVE accumulates k-tile slabs into s1, one
# tiny PE matmul per 128-token group reduces over partitions.
# The attention loop is software-pipelined: all partner-independent half-A
# work is emitted before any half-B work, covering the collectives.
#
# DMA-QUEUE PLAN (each dma_start on a HW-DGE ring costs ~0.65us of
# sequencer descriptor-gen + a ~2-3us serialized completion-receipt
# bubble, and the rings ramp slowly for the first ~15us, so BOTH the DMA
# count and which ring carries what were tuned empirically — many
# theoretically-prettier plans measured slower):
#   sync (SP HW-DGE):    wq/qx dt-pair slabs (alternating) -> vx ->
#                        recovery -> output
#   scalar (ACT HW-DGE): wq/qx dt-pair slabs (alternating) -> wv ->
#                        kT bounce -> evictions
#   gpsimd (SW-DGE):     b3 -> raw-K own half; + the two collective
#                        triggers (nothing compute-critical parks behind
#                        a collective)
# The Q'-phase inputs (wq+qx, 4MB) alternate dt-pair slabs across BOTH
# rings in matched dt order, so each ring carries 2MB and the last slab
# lands ~10us earlier than a ring-per-tensor plan (first MM ~15us, Q'
# stalls ~7us vs ~20us/~9us before).
# Output is bf16, one full-D DMA per 128-token tile (2KB rows, half the
# receipt bubbles); host upcasts and adds bo' = bv@Wo + bo (softmax rows
# sum to 1).
#
# Tried and rejected: fp8/DoubleRow everywhere (softmax-weighted averaging
# passes element noise ~1:1 to the output; every fp8 placement measured
# 2.7e-2..8.6e-2 rel err vs the 2e-2 gate in numpy pipeline sims),
# direct collective on the kx input (walrus verifier rejects ExternalInput
# collectives), HAM warm-up dummy matmuls (net-negative on hardware),
# cross-ring half splits and single-DMA-per-tensor plans (slower).

import functools

import ml_dtypes
import numpy as np

B, S, D = 4, 2048, 1024
N_CORES = 8
P = 128
DT = D // P        # 8 d-tiles of 128
TQ = S // 2        # 1024 query rows / kv-half rows per core
NQ = TQ // P       # 8 q-tiles
NK = S // P        # 16 k-tiles
NKH = NK // 2      # 8 k-tiles per half
SCALE = 1.0 / np.sqrt(np.float32(D))  # 1/32
BF16 = ml_dtypes.bfloat16
PAIRS = [[0, 1], [2, 3], [4, 5], [6, 7]]


@functools.lru_cache(maxsize=1)
def _build():
    import concourse.bass as bass  # noqa: F401  (registers engines)
    import concourse.mybir as mybir
    import concourse.tile as tile
    from concourse import bacc

    f32 = mybir.dt.float32
    bf16 = mybir.dt.bfloat16

    nc = bacc.Bacc("TRN2", target_bir_lowering=False, debug=False,
                   num_devices=N_CORES)

    qx = nc.dram_tensor("qx", [P, DT * TQ], bf16, kind="ExternalInput")
    kx = nc.dram_tensor("kx", [P, DT * TQ], bf16, kind="ExternalInput")
    vx = nc.dram_tensor("vx", [P, DT * TQ], bf16, kind="ExternalInput")
    wq = nc.dram_tensor("wq", [P, DT * D], bf16, kind="ExternalInput")
    wv = nc.dram_tensor("wv", [P, DT * D], bf16, kind="ExternalInput")
    b3_d = nc.dram_tensor("b3", [P, NK], f32, kind="ExternalInput")
    out_d = nc.dram_tensor("out", [TQ, D], bf16, kind="ExternalOutput")

    Ident = mybir.ActivationFunctionType.Identity
    Exp = mybir.ActivationFunctionType.Exp

    with tile.TileContext(nc) as tc:
        with (
            tc.tile_pool(name="const", bufs=1) as const,
            tc.tile_pool(name="wpool", bufs=2) as wpool,
            tc.tile_pool(name="big", bufs=1) as big,
            tc.tile_pool(name="work", bufs=2) as work,
            tc.tile_pool(name="sums", bufs=1) as sums,
            tc.tile_pool(name="ptp", bufs=2) as ptp,
            tc.tile_pool(name="dram", bufs=1, space="DRAM") as dram,
            tc.tile_pool(name="mmps", bufs=4, space="PSUM") as mmps,
            tc.tile_pool(name="scps", bufs=3, space="PSUM") as scps,
            tc.tile_pool(name="rsps", bufs=1, space="PSUM") as rsps,
        ):
            qx_ap = qx.ap().rearrange("p (dt t) -> p dt t", dt=DT)
            kx_ap = kx.ap().rearrange("p (dt t) -> p dt t", dt=DT)
            vx_ap = vx.ap().rearrange("p (dt t) -> p dt t", dt=DT)
            wq_ap = wq.ap().rearrange("p (dt n) -> p dt n", dt=DT)
            wv_ap = wv.ap().rearrange("p (dt n) -> p dt n", dt=DT)

            # ---- constants ----
            ones_sb = const.tile([P, 1], bf16, tag="ones")
            nc.vector.memset(ones_sb[:], 1.0)
            rs_ps = rsps.tile([P, NQ], f32, tag="rs")
            b3_sb = const.tile([P, NK], f32, tag="b3")
            # b3 rides gpsimd: every dma on a HW-DGE ring costs ~2-3us of
            # serialized completion-receipt, so the scalar ring head is
            # reserved for wq (which gates the first matmul).
            nc.gpsimd.dma_start(b3_sb[:], b3_d.ap())
            r_all = const.tile([P, NQ], f32, tag="rall")

            # ---- persistent intermediates ----
            QT = big.tile([P, DT, TQ], bf16, tag="QT")       # 2 MB  Q' = q@W'
            KTr = big.tile([P, DT, S], bf16, tag="KTr")      # 4 MB raw k, A+B
            Vtm = big.tile([P, NK, D], bf16, tag="Vtm")      # 4 MB token-major V'
            aoM = big.tile([P, NQ, D], bf16, tag="aoM")      # 2 MB attn_out half-A
            qxt = big.tile([P, DT, TQ], bf16, tag="qxt")     # 2 MB raw q slabs
            vxt = big.tile([P, DT, TQ], bf16, tag="vxt")     # 2 MB raw v slabs

            # ---- collectives: one 2MB AllReduce per tensor. K exchanges
            # RAW input data — a DRAM->DRAM bounce of the kx input (the
            # walrus verifier rejects collectives reading ExternalInputs
            # directly) rides the scalar queue tail so its 2MB never
            # blocks the sync descriptor ring the first matmuls wait on.
            ex_k_in = dram.tile([P, DT * TQ], bf16)
            ex_k_out = dram.tile([P, DT * TQ], bf16)
            ex_v_in = dram.tile([P, NKH, D], bf16)
            ex_v_out = dram.tile([P, NKH, D], bf16)

            # ---- weights/input streams: the empirically-tuned queue plan
            # from the header. qx slabs ride sync and wq slabs ride scalar
            # so the two HW-DGE rings deliver the Q'-phase inputs in
            # parallel; slab sizes 0.25/0.75/1MB balance earliest-start
            # (small first slab) against per-DMA receipt bubbles (few
            # DMAs). All slices are >=2KB-contiguous per partition.
            w_qk = wpool.tile([P, DT, D], bf16, tag="w")
            w_v = wpool.tile([P, DT, D], bf16, tag="w")
            # dt-pair slabs of wq/qx alternate across BOTH rings in
            # matched dt order, so each ring carries only 2MB of the
            # Q'-phase inputs and the last slab lands ~10us earlier than
            # a single-ring-per-tensor plan.
            nc.scalar.dma_start(w_qk[:, 0:2, :], wq_ap[:, 0:2, :])
            nc.sync.dma_start(qxt[:, 0:2, :], qx_ap[:, 0:2, :])
            nc.sync.dma_start(w_qk[:, 2:4, :], wq_ap[:, 2:4, :])
            nc.scalar.dma_start(qxt[:, 2:4, :], qx_ap[:, 2:4, :])
            nc.scalar.dma_start(w_qk[:, 4:6, :], wq_ap[:, 4:6, :])
            nc.sync.dma_start(qxt[:, 4:6, :], qx_ap[:, 4:6, :])
            nc.sync.dma_start(w_qk[:, 6:8, :], wq_ap[:, 6:8, :])
            nc.scalar.dma_start(qxt[:, 6:8, :], qx_ap[:, 6:8, :])
            nc.scalar.dma_start(w_v[:, :4, :], wv_ap[:, :4, :])
            nc.scalar.dma_start(w_v[:, 4:, :], wv_ap[:, 4:, :])
            nc.scalar.dma_start(ex_k_in[:], kx.ap())
            nc.sync.dma_start(vxt[:, :4, :], vx_ap[:, :4, :])
            nc.sync.dma_start(vxt[:, 4:, :], vx_ap[:, 4:, :])
            # raw K own half into SBUF (gpsimd SW-DGE: 2MB, needed late)
            nc.gpsimd.dma_start(KTr[:, :, 0:TQ], kx_ap[:, :, :])
            nc.gpsimd.collective_compute(
                "AllReduce", mybir.AluOpType.add, replica_groups=PAIRS,
                ins=[ex_k_in.opt()], outs=[ex_k_out.opt()],
            )

            # ---- Q' projection, dt-OUTER: for each (o-chunk, t-chunk)
            # combo, accumulate all 8 dt slabs into 4 PSUM banks. The
            # first matmul needs only the dt0 slab pair (~0.5MB landed).
            for tc_i in range(2):
                for och in range(2):
                    tsl = slice(tc_i * 512, (tc_i + 1) * 512)
                    pss = [mmps.tile([P, 512], f32, tag="mm",
                                     name=f"qp{och}{tc_i}{oi}")
                           for oi in range(4)]
                    for dt_i in range(DT):
                        for oi in range(4):
                            o = och * 4 + oi
                            nc.tensor.matmul(
                                pss[oi][:],
                                w_qk[:, dt_i, o * P:(o + 1) * P],
                                qxt[:, dt_i, tsl],
                                start=(dt_i == 0),
                                stop=(dt_i == DT - 1),
                            )
                    for oi in range(4):
                        nc.scalar.copy(QT[:, och * 4 + oi, tsl], pss[oi][:])

            # ---- V projection (own half, token-major -> Vtm[:, 0:8, :]) ----
            def v_proj():
                for c in range(TQ // 512):
                    for sub in range(4):        # 4 tok-tiles per chunk
                        tt = c * 4 + sub
                        for dc in range(2):     # dout chunks of 512
                            ps = mmps.tile([P, 512], f32, tag="mm")
                            for dt_i in range(DT):
                                nc.tensor.matmul(
                                    ps[:],
                                    vxt[:, dt_i, tt * P:(tt + 1) * P],
                                    w_v[:, dt_i, dc * 512:(dc + 1) * 512],
                                    start=(dt_i == 0),
                                    stop=(dt_i == DT - 1),
                                )
                            nc.scalar.copy(
                                Vtm[:, tt, dc * 512:(dc + 1) * 512], ps[:])
                    # stream the finished 4-tile group back in one DMA
                    nc.scalar.dma_start(ex_v_in[:, c * 4:(c + 1) * 4, :],
                                        Vtm[:, c * 4:(c + 1) * 4, :])
                nc.gpsimd.collective_compute(
                    "AllReduce", mybir.AluOpType.add, replica_groups=PAIRS,
                    ins=[ex_v_in.opt()], outs=[ex_v_out.opt()],
                )

            # partner-half recovery: DMA the pair-sum into the B-half
            # (sync queue tail), subtract own in place on the DVE with LATE
            # priority so the scheduler never parks these at the DVE queue
            # head in front of the s1/accumulation chains.
            ex_k_ap = ex_k_out[:].rearrange("p (dt t) -> p dt t", dt=DT)

            def sub_k(c):
                dst = KTr[:, :, TQ + c * 512:TQ + (c + 1) * 512]
                nc.sync.dma_start(dst, ex_k_ap[:, :, c * 512:(c + 1) * 512])
                with tc.high_priority(offset=-100000):
                    nc.vector.tensor_tensor(
                        dst, dst, KTr[:, :, c * 512:(c + 1) * 512],
                        mybir.AluOpType.subtract,
                    )

            def sub_v(c):
                dst = Vtm[:, NKH + c * 4:NKH + (c + 1) * 4, :]
                nc.sync.dma_start(dst, ex_v_out[:, c * 4:(c + 1) * 4, :])
                with tc.high_priority(offset=-100000):
                    nc.vector.tensor_tensor(
                        dst, dst, Vtm[:, c * 4:(c + 1) * 4, :],
                        mybir.AluOpType.subtract,
                    )

            # ---- attention, software-pipelined over 512-wide q-blocks ----
            pT_tiles = {}
            s1_tiles = {}

            def s1_add(blk, kt):
                s1 = s1_tiles[blk]
                if kt == 0:
                    nc.vector.tensor_copy(s1[:], pT_tiles[blk][:, 0, :])
                else:
                    nc.vector.tensor_tensor(
                        s1[:], pT_tiles[blk][:, kt, :], s1[:],
                        mybir.AluOpType.add)

            def score_half(blk, half, defer_sums=False):
                qsl = slice(blk * 512, (blk + 1) * 512)
                pT = pT_tiles[blk]
                for kt in range(half * NKH, half * NKH + NKH):
                    sc = scps.tile([P, 512], f32, tag="sc")
                    for dt_i in range(DT):
                        nc.tensor.matmul(
                            sc[:],
                            KTr[:, dt_i, kt * P:(kt + 1) * P],
                            QT[:, dt_i, qsl],
                            start=(dt_i == 0),
                            stop=(dt_i == DT - 1),
                        )
                    nc.scalar.activation(pT[:, kt, :], sc[:], Exp,
                                         bias=b3_sb[:, kt:kt + 1])
                    if not defer_sums:
                        s1_add(blk, kt)

            def row_sums(blk):
                s1 = s1_tiles[blk]
                for t4 in range(4):
                    nc.tensor.matmul(
                        rs_ps[:, blk * 4 + t4:blk * 4 + t4 + 1],
                        s1[:, t4 * P:(t4 + 1) * P],
                        ones_sb[:, 0:1],
                        start=True, stop=True,
                    )
                nc.vector.reciprocal(r_all[:, blk * 4:(blk + 1) * 4],
                                     rs_ps[:, blk * 4:(blk + 1) * 4])

            def attn_v(blk, half):
                pT = pT_tiles[blk]
                for tt4 in range(4):
                    tt = blk * 4 + tt4
                    fin = None
                    for dc in range(2):
                        if half == 1 and (tt4 * 2 + dc) % 2 == 0:
                            av = scps.tile([P, 512], f32, tag="sc", name="avs")
                        else:
                            av = mmps.tile([P, 512], f32, tag="mm", name="avm")
                        for kt in range(half * NKH, half * NKH + NKH):
                            nc.tensor.matmul(
                                av[:],
                                pT[:, kt, tt4 * P:(tt4 + 1) * P],
                                Vtm[:, kt, dc * 512:(dc + 1) * 512],
                                start=(kt == half * NKH),
                                stop=(kt == half * NKH + NKH - 1),
                            )
                        dsl = slice(dc * 512, (dc + 1) * 512)
                        if half == 0:
                            nc.scalar.copy(aoM[:, tt, dsl], av[:])
                        else:
                            if dc == 0:
                                fin = work.tile([P, D], bf16, tag="fin")
                            wf = work.tile([P, 512], f32, tag="wf")
                            nc.vector.tensor_tensor(
                                wf[:], av[:], aoM[:, tt, dsl],
                                mybir.AluOpType.add,
                            )
                            nc.scalar.activation(
                                fin[:, dsl], wf[:], Ident,
                                scale=r_all[:, tt:tt + 1],
                            )
                            if dc == 1:
                                # one full-D output DMA per token tile:
                                # 2KB-contiguous rows, half the per-DMA
                                # receipt bubbles on the sync ring
                                nc.sync.dma_start(
                                    out_d.ap()[tt * P:(tt + 1) * P, :],
                                    fin[:])

            def p1a(blk):
                pT_tiles[blk] = ptp.tile(
                    [P, NK, 512], bf16, tag="pT", name=f"pT{blk}")
                s1_tiles[blk] = sums.tile([P, 512], bf16, tag=f"s1b{blk}",
                                          name=f"s1b{blk}")
                score_half(blk, 0)

            def p2(blk):
                score_half(blk, 1)

            def p3(blk):
                attn_v(blk, 1)

            v_proj()
            p1a(0)
            attn_v(0, 0)
            p1a(1)
            attn_v(1, 0)
            sub_k(0)
            sub_k(1)
            sub_v(0)
            sub_v(1)
            p2(0)
            p2(1)
            row_sums(0)
            p3(0)
            row_sums(1)
            p3(1)

    nc.compile()
    return nc


def _numpy_reference(query, key, value, mask, Wq, bq, Wk, bk, Wv, bv, Wo, bo):
    q = query @ Wq + bq
    k = key @ Wk + bk
    v = value @ Wv + bv
    s = np.einsum("bsd,btd->bst", q, k) / np.sqrt(np.float32(q.shape[-1]))
    s = np.where(mask == 0, np.float32(-1e9), s)
    s = s - s.max(axis=-1, keepdims=True)
    e = np.exp(s)
    p = e / e.sum(axis=-1, keepdims=True)
    x = np.einsum("bst,btd->bsd", p, v)
    return (x @ Wo + bo).astype(np.float32)


def _pmajor(arr_T):
    # [D, T] feature-major -> partition-major [P, DT*T]
    T = arr_T.shape[1]
    return np.ascontiguousarray(
        arr_T.reshape(DT, P, T).transpose(1, 0, 2).reshape(P, DT * T))


def kernel(query, key, value, mask, Wq, bq, Wk, bk, Wv, bv, Wo, bo):
    query = np.asarray(query, np.float32)
    key = np.asarray(key, np.float32)
    value = np.asarray(value, np.float32)
    mask = np.asarray(mask)
    if not np.all(mask != 0):
        # This problem's mask is always all-ones; keep a correct fallback.
        return _numpy_reference(query, key, value, mask, Wq, bq, Wk, bk,
                                Wv, bv, Wo, bo)

    from concourse.bass_utils import run_bass_kernel_spmd

    nc = _build()

    Wq32 = np.asarray(Wq, np.float32)
    Wk32 = np.asarray(Wk, np.float32)
    # K projection fused into Q: W' = (Wq Wk^T)/sqrt(D), exact in fp32
    wq_b = ((Wq32 @ Wk32.T) * SCALE).astype(BF16)
    # output projection fused into V: V' = v @ (Wv Wo), exact in fp32
    wv_b = (np.asarray(Wv, np.float32) @ np.asarray(Wo, np.float32)
            ).astype(BF16)
    # fusion leftover: per-k score bias b3[k] = k.(Wk bq)/sqrt(D) (the
    # per-q terms cancel in softmax); zero here since bq = 0.
    wkbq = (Wk32 @ np.asarray(bq, np.float32)) * SCALE
    b3_full = np.asarray(key, np.float32) @ wkbq          # [B, S]
    bo_eff = (np.asarray(bv, np.float32) @ np.asarray(Wo, np.float32)
              + np.asarray(bo, np.float32)).astype(np.float32)

    wq_pm = _pmajor(wq_b)
    wv_pm = _pmajor(wv_b)
    in_maps = []
    for c in range(N_CORES):
        b, h = divmod(c, 2)
        sl = slice(h * TQ, (h + 1) * TQ)
        sl_p = slice((1 - h) * TQ, (2 - h) * TQ)
        b3_core = np.concatenate([b3_full[b, sl], b3_full[b, sl_p]])
        in_maps.append({
            "qx": _pmajor(query[b, sl].T.astype(BF16)),
            "kx": _pmajor(key[b, sl].T.astype(BF16)),
            "vx": _pmajor(value[b, sl].T.astype(BF16)),
            "wq": wq_pm, "wv": wv_pm,
            "b3": np.ascontiguousarray(
                b3_core.reshape(NK, P).T.astype(np.float32)),
        })

    global _last_in_maps
    _last_in_maps = in_maps
    res = run_bass_kernel_spmd(nc, in_maps, list(range(N_CORES)))

    out = np.empty((B, S, D), np.float32)
    for c in range(N_CORES):
        b, h = divmod(c, 2)
        out[b, h * TQ:(h + 1) * TQ] = res.results[c]["out"].astype(np.float32)
    out += bo_eff
    return out


# revision 47
# speedup vs baseline: 1.0383x; 1.0009x over previous
# Trainium2 Bass kernel for single-head attention:
#   out = softmax((q@Wq+bq)(k@Wk+bk)^T / sqrt(D)) @ (v@Wv+bv) @ Wo + bo
# Full shapes: query/key/value [4, 2048, 1024], D=1024, mask all-ones.
#
# Sharding: data-parallel over (batch, query-half) -> 8 shards, one per
# NeuronCore. Core c handles batch b=c//2, query rows [h*1024, (h+1)*1024)
# with h=c%2. Each core projects only its OWN half of the batch's key/value
# tokens. The k-axis is PERMUTED per core: own tokens occupy k-positions
# [0, 1024) ("half A"), partner tokens [1024, 2048) ("half B") — softmax and
# P@V are permutation-invariant as long as K and V use the same order, so
# the SPMD program stays rank-independent. The partner half is obtained
# with a pairwise AllReduce(add) of the bf16 halves plus an on-chip
# subtract (partner = sum - own), which keeps every access pattern static.
#
# KEY FUSIONS (host-side exact fp32 weight algebra, device work deleted):
# 1. Output projection folded into V:
#      (P @ (v Wv)) @ Wo == P @ (v (Wv Wo)),  Wvo = Wv@Wo free on host.
# 2. K projection folded into Q:
#      (q Wq + bq)·(k Wk + bk) = q (Wq Wk^T) k^T + [softmax-invariant terms]
#    W' = (Wq Wk^T)/sqrt(D) on host; scores contract RAW k against
#    Q' = q@W'; leftover per-k bias b3[k] rides the Exp eviction's ACT bias.
#    This lets the K exchange run on RAW input data, so the collective
#    fires with no compute dependency at all.
#
# LAYOUT: everything feature-major; ALL host inputs are PARTITION-MAJOR
# [128, dt*cols] so every DMA descriptor is a contiguous >=2KB
# per-partition line (the naive [D, T] layout yields 512B descriptors and
# the DMA system crawls at ~30GB/s; partition-major sustains HW-DGE rate).
#   qx/kx/vx [P, DT*TQ]   (x[p, dt*TQ+t] = x_T[dt*128+p, t]),  bf16
#   wq/wv    [P, DT*D]    (w[p, dt*D+n] = W[dt*128+p, n]),     bf16
# Q' projection runs dt-OUTER (accumulating 4 o-banks per (och, tc) combo,
# tc-outer) so compute starts once the first per-dt slab pair (~0.5MB)
# lands instead of the full 4MB.
# Scores^T[k,q] accumulate over d (lhsT = raw-k tile, rhs = Q' block); Exp
# evicts P^T unnormalized; 1/rowsum applied as per-partition ACT scale on
# the final eviction. Row sums: DVE accumulates k-tile slabs into s1, one
# tiny PE matmul per 128-token group reduces over partitions.
# The attention loop is software-pipelined: all partner-independent half-A
# work is emitted before any half-B work, covering the collectives.
#
# DMA-QUEUE PLAN (each dma_start on a HW-DGE ring costs ~0.65us of
# sequencer descriptor-gen + a ~2-3us serialized completion-receipt
# bubble, and the rings ramp slowly for the first ~15us, so BOTH the DMA
# count and which ring carries what were tuned empirically — many
# theoretically-prettier plans measured slower):
#   sync (SP HW-DGE):    wq/qx dt-pair slabs (alternating) -> vx ->
#                        recovery -> output
#   scalar (ACT HW-DGE): wq/qx dt-pair slabs (alternating) -> wv ->
#                        kT bounce -> evictions
#   gpsimd (SW-DGE):     b3 -> raw-K own half; + the two collective
#                        triggers (nothing compute-critical parks behind
#                        a collective)
# The Q'-phase inputs (wq+qx, 4MB) alternate dt-pair slabs across BOTH
# rings in matched dt order, so each ring carries 2MB and the last slab
# lands ~10us earlier than a ring-per-tensor plan (first MM ~15us, Q'
# stalls ~7us vs ~20us/~9us before).
# Output is bf16, one full-D DMA per 128-token tile (2KB rows, half the
# receipt bubbles); host upcasts and adds bo' = bv@Wo + bo (softmax rows
# sum to 1).
#
# Tried and rejected: fp8/DoubleRow everywhere (softmax-weighted averaging
# passes element noise ~1:1 to the output; every fp8 placement measured
# 2.7e-2..8.6e-2 rel err vs the 2e-2 gate in numpy pipeline sims),
# direct collective on the kx input (walrus verifier rejects ExternalInput
# collectives), HAM warm-up dummy matmuls (net-negative on hardware),
# cross-ring half splits and single-DMA-per-tensor plans (slower).

import functools

import ml_dtypes
import numpy as np

B, S, D = 4, 2048, 1024
N_CORES = 8
P = 128
DT = D // P        # 8 d-tiles of 128
TQ = S // 2        # 1024 query rows / kv-half rows per core
NQ = TQ // P       # 8 q-tiles
NK = S // P        # 16 k-tiles
NKH = NK // 2      # 8 k-tiles per half
SCALE = 1.0 / np.sqrt(np.float32(D))  # 1/32
BF16 = ml_dtypes.bfloat16
PAIRS = [[0, 1], [2, 3], [4, 5], [6, 7]]


@functools.lru_cache(maxsize=1)
def _build():
    import concourse.bass as bass  # noqa: F401  (registers engines)
    import concourse.mybir as mybir
    import concourse.tile as tile
    from concourse import bacc

    f32 = mybir.dt.float32
    bf16 = mybir.dt.bfloat16

    nc = bacc.Bacc("TRN2", target_bir_lowering=False, debug=False,
                   num_devices=N_CORES)

    qx = nc.dram_tensor("qx", [P, DT * TQ], bf16, kind="ExternalInput")
    kx = nc.dram_tensor("kx", [P, DT * TQ], bf16, kind="ExternalInput")
    vx = nc.dram_tensor("vx", [P, DT * TQ], bf16, kind="ExternalInput")
    wq = nc.dram_tensor("wq", [P, DT * D], bf16, kind="ExternalInput")
    wv = nc.dram_tensor("wv", [P, DT * D], bf16, kind="ExternalInput")
    b3_d = nc.dram_tensor("b3", [P, NK], f32, kind="ExternalInput")
    out_d = nc.dram_tensor("out", [TQ, D], bf16, kind="ExternalOutput")

    Ident = mybir.ActivationFunctionType.Identity
    Exp = mybir.ActivationFunctionType.Exp

    with tile.TileContext(nc) as tc:
        with (
            tc.tile_pool(name="const", bufs=1) as const,
            tc.tile_pool(name="wpool", bufs=2) as wpool,
            tc.tile_pool(name="big", bufs=1) as big,
            tc.tile_pool(name="work", bufs=3) as work,
            tc.tile_pool(name="sums", bufs=1) as sums,
            tc.tile_pool(name="ptp", bufs=2) as ptp,
            tc.tile_pool(name="dram", bufs=1, space="DRAM") as dram,
            tc.tile_pool(name="mmps", bufs=4, space="PSUM") as mmps,
            tc.tile_pool(name="scps", bufs=3, space="PSUM") as scps,
            tc.tile_pool(name="rsps", bufs=1, space="PSUM") as rsps,
        ):
            qx_ap = qx.ap().rearrange("p (dt t) -> p dt t", dt=DT)
            kx_ap = kx.ap().rearrange("p (dt t) -> p dt t", dt=DT)
            vx_ap = vx.ap().rearrange("p (dt t) -> p dt t", dt=DT)
            wq_ap = wq.ap().rearrange("p (dt n) -> p dt n", dt=DT)
            wv_ap = wv.ap().rearrange("p (dt n) -> p dt n", dt=DT)

            # ---- constants ----
            ones_sb = const.tile([P, 1], bf16, tag="ones")
            nc.vector.memset(ones_sb[:], 1.0)
            rs_ps = rsps.tile([P, NQ], f32, tag="rs")
            b3_sb = const.tile([P, NK], f32, tag="b3")
            # b3 rides gpsimd: every dma on a HW-DGE ring costs ~2-3us of
            # serialized completion-receipt, so the scalar ring head is
            # reserved for wq (which gates the first matmul).
            nc.gpsimd.dma_start(b3_sb[:], b3_d.ap())
            r_all = const.tile([P, NQ], f32, tag="rall")

            # ---- persistent intermediates ----
            QT = big.tile([P, DT, TQ], bf16, tag="QT")       # 2 MB  Q' = q@W'
            KTr = big.tile([P, DT, S], bf16, tag="KTr")      # 4 MB raw k, A+B
            Vtm = big.tile([P, NK, D], bf16, tag="Vtm")      # 4 MB token-major V'
            aoM = big.tile([P, NQ, D], bf16, tag="aoM")      # 2 MB attn_out half-A
            qxt = big.tile([P, DT, TQ], bf16, tag="qxt")     # 2 MB raw q slabs
            vxt = big.tile([P, DT, TQ], bf16, tag="vxt")     # 2 MB raw v slabs

            # ---- collectives: one 2MB AllReduce per tensor. K exchanges
            # RAW input data — a DRAM->DRAM bounce of the kx input (the
            # walrus verifier rejects collectives reading ExternalInputs
            # directly) rides the scalar queue tail so its 2MB never
            # blocks the sync descriptor ring the first matmuls wait on.
            ex_k_in = dram.tile([P, DT * TQ], bf16)
            ex_k_out = dram.tile([P, DT * TQ], bf16)
            ex_v_in = dram.tile([P, NKH, D], bf16)
            ex_v_out = dram.tile([P, NKH, D], bf16)

            # ---- weights/input streams: the empirically-tuned queue plan
            # from the header. qx slabs ride sync and wq slabs ride scalar
            # so the two HW-DGE rings deliver the Q'-phase inputs in
            # parallel; slab sizes 0.25/0.75/1MB balance earliest-start
            # (small first slab) against per-DMA receipt bubbles (few
            # DMAs). All slices are >=2KB-contiguous per partition.
            w_qk = wpool.tile([P, DT, D], bf16, tag="w")
            w_v = wpool.tile([P, DT, D], bf16, tag="w")
            # dt-pair slabs of wq/qx alternate across BOTH rings in
            # matched dt order, so each ring carries only 2MB of the
            # Q'-phase inputs and the last slab lands ~10us earlier than
            # a single-ring-per-tensor plan.
            nc.scalar.dma_start(w_qk[:, 0:2, :], wq_ap[:, 0:2, :])
            nc.sync.dma_start(qxt[:, 0:2, :], qx_ap[:, 0:2, :])
            nc.sync.dma_start(w_qk[:, 2:4, :], wq_ap[:, 2:4, :])
            nc.scalar.dma_start(qxt[:, 2:4, :], qx_ap[:, 2:4, :])
            nc.scalar.dma_start(w_qk[:, 4:6, :], wq_ap[:, 4:6, :])
            nc.sync.dma_start(qxt[:, 4:6, :], qx_ap[:, 4:6, :])
            nc.sync.dma_start(w_qk[:, 6:8, :], wq_ap[:, 6:8, :])
            nc.scalar.dma_start(qxt[:, 6:8, :], qx_ap[:, 6:8, :])
            nc.scalar.dma_start(w_v[:, :4, :], wv_ap[:, :4, :])
            nc.scalar.dma_start(w_v[:, 4:, :], wv_ap[:, 4:, :])
            nc.scalar.dma_start(ex_k_in[:], kx.ap())
            nc.sync.dma_start(vxt[:, :4, :], vx_ap[:, :4, :])
            nc.sync.dma_start(vxt[:, 4:, :], vx_ap[:, 4:, :])
            # raw K own half into SBUF (gpsimd SW-DGE: 2MB, needed late)
            nc.gpsimd.dma_start(KTr[:, :, 0:TQ], kx_ap[:, :, :])
            nc.gpsimd.collective_compute(
                "AllReduce", mybir.AluOpType.add, replica_groups=PAIRS,
                ins=[ex_k_in.opt()], outs=[ex_k_out.opt()],
            )

            # ---- Q' projection, dt-OUTER: for each (o-chunk, t-chunk)
            # combo, accumulate all 8 dt slabs into 4 PSUM banks. The
            # first matmul needs only the dt0 slab pair (~0.5MB landed).
            for tc_i in range(2):
                for och in range(2):
                    tsl = slice(tc_i * 512, (tc_i + 1) * 512)
                    pss = [mmps.tile([P, 512], f32, tag="mm",
                                     name=f"qp{och}{tc_i}{oi}")
                           for oi in range(4)]
                    for dt_i in range(DT):
                        for oi in range(4):
                            o = och * 4 + oi
                            nc.tensor.matmul(
                                pss[oi][:],
                                w_qk[:, dt_i, o * P:(o + 1) * P],
                                qxt[:, dt_i, tsl],
                                start=(dt_i == 0),
                                stop=(dt_i == DT - 1),
                            )
                    for oi in range(4):
                        nc.scalar.copy(QT[:, och * 4 + oi, tsl], pss[oi][:])

            # ---- V projection (own half, token-major -> Vtm[:, 0:8, :]) ----
            def v_proj():
                for c in range(TQ // 512):
                    for sub in range(4):        # 4 tok-tiles per chunk
                        tt = c * 4 + sub
                        for dc in range(2):     # dout chunks of 512
                            ps = mmps.tile([P, 512], f32, tag="mm")
                            for dt_i in range(DT):
                                nc.tensor.matmul(
                                    ps[:],
                                    vxt[:, dt_i, tt * P:(tt + 1) * P],
                                    w_v[:, dt_i, dc * 512:(dc + 1) * 512],
                                    start=(dt_i == 0),
                                    stop=(dt_i == DT - 1),
                                )
                            nc.scalar.copy(
                                Vtm[:, tt, dc * 512:(dc + 1) * 512], ps[:])
                    # stream the finished 4-tile group back in one DMA
                    nc.scalar.dma_start(ex_v_in[:, c * 4:(c + 1) * 4, :],
                                        Vtm[:, c * 4:(c + 1) * 4, :])
                nc.gpsimd.collective_compute(
                    "AllReduce", mybir.AluOpType.add, replica_groups=PAIRS,
                    ins=[ex_v_in.opt()], outs=[ex_v_out.opt()],
                )

            # partner-half recovery: DMA the pair-sum into the B-half
            # (sync queue tail), subtract own in place on the DVE with LATE
            # priority so the scheduler never parks these at the DVE queue
            # head in front of the s1/accumulation chains.
            ex_k_ap = ex_k_out[:].rearrange("p (dt t) -> p dt t", dt=DT)

            def sub_k(c):
                dst = KTr[:, :, TQ + c * 512:TQ + (c + 1) * 512]
                nc.sync.dma_start(dst, ex_k_ap[:, :, c * 512:(c + 1) * 512])
                with tc.high_priority(offset=-100000):
                    nc.vector.tensor_tensor(
                        dst, dst, KTr[:, :, c * 512:(c + 1) * 512],
                        mybir.AluOpType.subtract,
                    )

            def sub_v(c):
                dst = Vtm[:, NKH + c * 4:NKH + (c + 1) * 4, :]
                nc.sync.dma_start(dst, ex_v_out[:, c * 4:(c + 1) * 4, :])
                with tc.high_priority(offset=-100000):
                    nc.vector.tensor_tensor(
                        dst, dst, Vtm[:, c * 4:(c + 1) * 4, :],
                        mybir.AluOpType.subtract,
                    )

            # ---- attention, software-pipelined over 512-wide q-blocks ----
            pT_tiles = {}
            s1_tiles = {}

            def s1_add(blk, kt):
                s1 = s1_tiles[blk]
                if kt == 0:
                    nc.vector.tensor_copy(s1[:], pT_tiles[blk][:, 0, :])
                else:
                    nc.vector.tensor_tensor(
                        s1[:], pT_tiles[blk][:, kt, :], s1[:],
                        mybir.AluOpType.add)

            def score_half(blk, half, defer_sums=False):
                qsl = slice(blk * 512, (blk + 1) * 512)
                pT = pT_tiles[blk]
                for kt in range(half * NKH, half * NKH + NKH):
                    sc = scps.tile([P, 512], f32, tag="sc")
                    for dt_i in range(DT):
                        nc.tensor.matmul(
                            sc[:],
                            KTr[:, dt_i, kt * P:(kt + 1) * P],
                            QT[:, dt_i, qsl],
                            start=(dt_i == 0),
                            stop=(dt_i == DT - 1),
                        )
                    nc.scalar.activation(pT[:, kt, :], sc[:], Exp,
                                         bias=b3_sb[:, kt:kt + 1])
                    if not defer_sums:
                        s1_add(blk, kt)

            def row_sums(blk):
                s1 = s1_tiles[blk]
                for t4 in range(4):
                    nc.tensor.matmul(
                        rs_ps[:, blk * 4 + t4:blk * 4 + t4 + 1],
                        s1[:, t4 * P:(t4 + 1) * P],
                        ones_sb[:, 0:1],
                        start=True, stop=True,
                    )
                nc.vector.reciprocal(r_all[:, blk * 4:(blk + 1) * 4],
                                     rs_ps[:, blk * 4:(blk + 1) * 4])

            def attn_v(blk, half):
                pT = pT_tiles[blk]
                for tt4 in range(4):
                    tt = blk * 4 + tt4
                    fin = None
                    for dc in range(2):
                        if half == 1 and (tt4 * 2 + dc) % 2 == 0:
                            av = scps.tile([P, 512], f32, tag="sc", name="avs")
                        else:
                            av = mmps.tile([P, 512], f32, tag="mm", name="avm")
                        for kt in range(half * NKH, half * NKH + NKH):
                            nc.tensor.matmul(
                                av[:],
                                pT[:, kt, tt4 * P:(tt4 + 1) * P],
                                Vtm[:, kt, dc * 512:(dc + 1) * 512],
                                start=(kt == half * NKH),
                                stop=(kt == half * NKH + NKH - 1),
                            )
                        dsl = slice(dc * 512, (dc + 1) * 512)
                        if half == 0:
                            nc.scalar.copy(aoM[:, tt, dsl], av[:])
                        else:
                            if dc == 0:
                                fin = work.tile([P, D], bf16, tag="fin")
                            wf = work.tile([P, 512], f32, tag="wf")
                            nc.vector.tensor_tensor(
                                wf[:], av[:], aoM[:, tt, dsl],
                                mybir.AluOpType.add,
                            )
                            nc.scalar.activation(
                                fin[:, dsl], wf[:], Ident,
                                scale=r_all[:, tt:tt + 1],
                            )
                            if dc == 1:
                                # one full-D output DMA per token tile:
                                # 2KB-contiguous rows, half the per-DMA
                                # receipt bubbles on the sync ring
                                nc.sync.dma_start(
                                    out_d.ap()[tt * P:(tt + 1) * P, :],
                                    fin[:])

            def p1a(blk):
                pT_tiles[blk] = ptp.tile(
                    [P, NK, 512], bf16, tag="pT", name=f"pT{blk}")
                s1_tiles[blk] = sums.tile([P, 512], bf16, tag=f"s1b{blk}",
                                          name=f"s1b{blk}")
                score_half(blk, 0)

            def p2(blk):
                score_half(blk, 1)

            def p3(blk):
                attn_v(blk, 1)

            v_proj()
            p1a(0)
            attn_v(0, 0)
            p1a(1)
            attn_v(1, 0)
            sub_k(0)
            sub_k(1)
            sub_v(0)
            sub_v(1)
            p2(0)
            p2(1)
            row_sums(0)
            p3(0)
            row_sums(1)
            p3(1)

    nc.compile()
    return nc


def _numpy_reference(query, key, value, mask, Wq, bq, Wk, bk, Wv, bv, Wo, bo):
    q = query @ Wq + bq
    k = key @ Wk + bk
    v = value @ Wv + bv
    s = np.einsum("bsd,btd->bst", q, k) / np.sqrt(np.float32(q.shape[-1]))
    s = np.where(mask == 0, np.float32(-1e9), s)
    s = s - s.max(axis=-1, keepdims=True)
    e = np.exp(s)
    p = e / e.sum(axis=-1, keepdims=True)
    x = np.einsum("bst,btd->bsd", p, v)
    return (x @ Wo + bo).astype(np.float32)


def _pmajor(arr_T):
    # [D, T] feature-major -> partition-major [P, DT*T]
    T = arr_T.shape[1]
    return np.ascontiguousarray(
        arr_T.reshape(DT, P, T).transpose(1, 0, 2).reshape(P, DT * T))


def kernel(query, key, value, mask, Wq, bq, Wk, bk, Wv, bv, Wo, bo):
    query = np.asarray(query, np.float32)
    key = np.asarray(key, np.float32)
    value = np.asarray(value, np.float32)
    mask = np.asarray(mask)
    if not np.all(mask != 0):
        # This problem's mask is always all-ones; keep a correct fallback.
        return _numpy_reference(query, key, value, mask, Wq, bq, Wk, bk,
                                Wv, bv, Wo, bo)

    from concourse.bass_utils import run_bass_kernel_spmd

    nc = _build()

    Wq32 = np.asarray(Wq, np.float32)
    Wk32 = np.asarray(Wk, np.float32)
    # K projection fused into Q: W' = (Wq Wk^T)/sqrt(D), exact in fp32
    wq_b = ((Wq32 @ Wk32.T) * SCALE).astype(BF16)
    # output projection fused into V: V' = v @ (Wv Wo), exact in fp32
    wv_b = (np.asarray(Wv, np.float32) @ np.asarray(Wo, np.float32)
            ).astype(BF16)
    # fusion leftover: per-k score bias b3[k] = k.(Wk bq)/sqrt(D) (the
    # per-q terms cancel in softmax); zero here since bq = 0.
    wkbq = (Wk32 @ np.asarray(bq, np.float32)) * SCALE
    b3_full = np.asarray(key, np.float32) @ wkbq          # [B, S]
    bo_eff = (np.asarray(bv, np.float32) @ np.asarray(Wo, np.float32)
              + np.asarray(bo, np.float32)).astype(np.float32)

    wq_pm = _pmajor(wq_b)
    wv_pm = _pmajor(wv_b)
    in_maps = []
    for c in range(N_CORES):
        b, h = divmod(c, 2)
        sl = slice(h * TQ, (h + 1) * TQ)
        sl_p = slice((1 - h) * TQ, (2 - h) * TQ)
        b3_core = np.concatenate([b3_full[b, sl], b3_full[b, sl_p]])
        in_maps.append({
            "qx": _pmajor(query[b, sl].T.astype(BF16)),
            "kx": _pmajor(key[b, sl].T.astype(BF16)),
            "vx": _pmajor(value[b, sl].T.astype(BF16)),
            "wq": wq_pm, "wv": wv_pm,
            "b3": np.ascontiguousarray(
                b3_core.reshape(NK, P).T.astype(np.float32)),
        })

    global _last_in_maps
    _last_in_maps = in_maps
    res = run_bass_kernel_spmd(nc, in_maps, list(range(N_CORES)))

    out = np.empty((B, S, D), np.float32)
    for c in range(N_CORES):
        b, h = divmod(c, 2)
        out[b, h * TQ:(h + 1) * TQ] = res.results[c]["out"].astype(np.float32)
    out += bo_eff
    return out
